# revision 2
# baseline (speedup 1.0000x reference)
"""Bass/Trainium2 SPMD kernel for BertUnpadSelfAttentionWithExtras.

Problem shape (hardcoded, matches the grading reference):
  B=4 batches, S=1024 max seqlen, H=12 heads, D=64 head dim, DIM=768,
  L=512 real tokens per sequence (NNZ=2048 total).

Sharding over 8 cores: core c handles batch b = c//2 and head group
g = c%2 (6 heads each). Fully data-parallel, no collectives.

Key insight: padded key positions (>=512 within each sequence) have
k = v = 0 (scatter leaves them zero) and bias ~= -10000, so
exp(score - anything) underflows to exactly 0.0 in fp32 -> they
contribute nothing to softmax numerator or denominator. We therefore
compute attention over only the first 512 keys and read only
bias[:, :, :512, :512].

Device layout (per core):
  hsT  [768, 512]  : hidden states of this batch, transposed (host prep)
  wT   [768, 1152] : W^T columns for this head group: [q(384)|k(384)|v(384)],
                     q columns pre-scaled by 1/sqrt(64) (host prep)
  bvec [1, 1152]   : qkv bias slice (q part pre-scaled), only if nonzero
  biasT[6, 512, 512]: additive attn bias, transposed to [h, k, q] (host prep)
  out  [512, 384]  : output rows (tokens) x (6 heads * 64)

  qT/kT computed as [feat, tok] tiles -> directly usable as matmul
  lhsT/rhs for scoresT[k, q] = k @ qT. exp(scoresT) tiles are directly
  the lhsT for attn = probsT.T @ v_aug, where v_aug has a ones column
  per head giving the softmax denominator in the same PSUM tile.
"""

import numpy as np
from contextlib import ExitStack

import concourse.bass as bass
import concourse.mybir as mybir
import concourse.tile as tile
from concourse.bass_utils import run_bass_kernel_spmd

N_CORES = 8
B, S, H, D = 4, 1024, 12, 64
DIM = H * D          # 768
L = 512              # real tokens per sequence
G = 2                # head groups per batch
HPG = H // G         # 6 heads per group
FEAT = HPG * D       # 384 features per group
HID = DIM            # 768 contraction dim
KC = HID // 128      # 6 hidden chunks
TC = L // 128        # 4 token chunks
F32 = mybir.dt.float32

_PROGRAM_CACHE: dict = {}


def _split_multiwaits(nc):
    """This walrus build rejects >1 sync wait per instruction; hoist all
    but the last wait onto single-wait NoOps preceding the instruction."""
    for f in nc.m.functions:
        for bb in f.blocks:
            insts = bb.instructions
            new = []
            changed = False
            for inst in insts:
                si = inst.sync_info
                waits = list(si.on_wait) if si and si.on_wait else []
                if len(waits) > 1:
                    changed = True
                    for j, w in enumerate(waits[:-1]):
                        new.append(mybir.InstNoOp(
                            name=f"{inst.name}-waitsplit-{j}",
                            engine=inst.engine,
                            sync_info=mybir.SyncInfo(on_wait=[w], on_update=[]),
                        ))
                    si.on_wait = [waits[-1]]
                new.append(inst)
            if changed:
                try:
                    bb.instructions = new
                except Exception:
                    insts.clear()
                    insts.extend(new)


def _emit_body(ctx, nc, tc, hsT_d, wT_d, biasT_d, out_d, bvec_d, uid):
    Exp = mybir.ActivationFunctionType.Exp
    has_bias = bvec_d is not None

    pool = ctx.enter_context(tc.tile_pool(name=f"sb{uid}", bufs=1))
    bias_pool = ctx.enter_context(tc.tile_pool(name=f"bias{uid}", bufs=6))
    out_pool = ctx.enter_context(tc.tile_pool(name=f"out{uid}", bufs=3))
    psum_qkv = ctx.enter_context(tc.tile_pool(name=f"pq{uid}", bufs=3, space="PSUM"))
    psum_sc = ctx.enter_context(tc.tile_pool(name=f"ps{uid}", bufs=3, space="PSUM"))
    psum_at = ctx.enter_context(tc.tile_pool(name=f"pa{uid}", bufs=2, space="PSUM"))

    # --- load inputs ---
    wt = [pool.tile([128, 3 * FEAT], F32, tag=f"w{k}", name=f"w{k}") for k in range(KC)]
    for k in range(KC):
        nc.sync.dma_start(out=wt[k][:], in_=wT_d[k * 128:(k + 1) * 128, :])
    hst = [pool.tile([128, L], F32, tag=f"h{k}", name=f"h{k}") for k in range(KC)]
    for k in range(KC):
        nc.sync.dma_start(out=hst[k][:], in_=hsT_d[k * 128:(k + 1) * 128, :])
    if has_bias:
        bvec = pool.tile([1, 3 * FEAT], F32, tag="bvec", name="bvec")
        nc.sync.dma_start(out=bvec[:], in_=bvec_d[:])
        ones = pool.tile([1, L], F32, tag="ones", name="ones")
        nc.vector.memset(ones[:], 1.0)

    # --- QKV projection ---
    # qT/kT: [feat, tok] tiles (3 each of [128, 512]; 2 heads per tile)
    qkt = []  # [q0,q1,q2,k0,k1,k2]
    for which in range(2):  # 0=q, 1=k
        for m in range(FEAT // 128):
            ps = psum_qkv.tile([128, L], F32, tag="pqkv", name="pqkv")
            col0 = which * FEAT + m * 128
            for k in range(KC):
                nc.tensor.matmul(
                    ps[:], lhsT=wt[k][:, col0:col0 + 128], rhs=hst[k][:],
                    start=(k == 0), stop=(k == KC - 1 and not has_bias))
            if has_bias:
                nc.tensor.matmul(
                    ps[:], lhsT=bvec[0:1, col0:col0 + 128], rhs=ones[0:1, :],
                    start=False, stop=True)
            sb = pool.tile([128, L], F32, tag=f"qk{which}{m}", name=f"qk{which}{m}")
            nc.scalar.copy(sb[:], ps[:])
            qkt.append(sb)
    qt, kt = qkt[:3], qkt[3:]

    # v in natural [tok, feat] layout, interleaved with a ones column per
    # head: v_aug[t] is [128, 6*65], columns h*65..h*65+63 = v_h, col h*65+64 = 1
    v_aug = []
    for t in range(TC):
        ps = psum_qkv.tile([128, FEAT], F32, tag="pqkv", name="pqkv_v")
        for k in range(KC):
            nc.tensor.matmul(
                ps[:], lhsT=hst[k][:, t * 128:(t + 1) * 128],
                rhs=wt[k][:, 2 * FEAT:3 * FEAT],
                start=(k == 0), stop=(k == KC - 1 and not has_bias))
        if has_bias:
            nc.tensor.matmul(
                ps[:], lhsT=ones[0:1, :128], rhs=bvec[0:1, 2 * FEAT:3 * FEAT],
                start=False, stop=True)
        va = pool.tile([128, HPG * (D + 1)], F32, tag=f"va{t}", name=f"va{t}")
        va3 = va[:].rearrange("p (h e) -> p h e", h=HPG)
        nc.vector.tensor_copy(
            va3[:, :, 0:D], ps[:].rearrange("p (h e) -> p h e", h=HPG))
        nc.vector.memset(va3[:, :, D:D + 1], 1.0)
        v_aug.append(va)

    # --- scoresT + softmax numerators ---
    # probs[h][kc]: [128(k), 512(q)] = exp(kT_chunk @ qT + biasT)
    probs = [[None] * TC for _ in range(HPG)]
    for h in range(HPG):
        ktile, part0 = kt[h // 2], (h % 2) * D
        qtile = qt[h // 2]
        for kc in range(TC):
            sc = psum_sc.tile([128, L], F32, tag="sc", name="sc")
            nc.tensor.matmul(
                sc[:],
                lhsT=ktile[part0:part0 + D, kc * 128:(kc + 1) * 128],
                rhs=qtile[part0:part0 + D, :],
                start=True, stop=True)
            bt = bias_pool.tile([128, L], F32, tag="bt", name="bt")
            nc.sync.dma_start(out=bt[:], in_=biasT_d[h, kc * 128:(kc + 1) * 128, :])
            nc.vector.tensor_add(sc[:], sc[:], bt[:])
            pr = pool.tile([128, L], F32, tag=f"pr{h}_{kc}", name=f"pr{h}_{kc}")
            nc.scalar.activation(pr[:], sc[:], Exp)
            probs[h][kc] = pr

    # --- attention: out[q, h*64+d] = (probsT.T @ v_aug) / denom ---
    for qc in range(TC):
        at = psum_at.tile([128, HPG * (D + 1)], F32, tag="at", name="at")
        for h in range(HPG):
            c0 = h * (D + 1)
            for kc in range(TC):
                nc.tensor.matmul(
                    at[:, c0:c0 + D + 1],
                    lhsT=probs[h][kc][:, qc * 128:(qc + 1) * 128],
                    rhs=v_aug[kc][:, c0:c0 + D + 1],
                    start=(kc == 0), stop=(kc == TC - 1))
        rc = out_pool.tile([128, HPG], F32, tag="rc", name="rc")
        for h in range(HPG):
            nc.vector.reciprocal(rc[:, h:h + 1], at[:, h * (D + 1) + D:h * (D + 1) + D + 1])
        ot = out_pool.tile([128, FEAT], F32, tag="ot", name="ot")
        for h in range(HPG):
            nc.vector.tensor_scalar_mul(
                ot[:, h * D:(h + 1) * D], at[:, h * (D + 1):h * (D + 1) + D],
                rc[:, h:h + 1])
        nc.sync.dma_start(out=out_d[qc * 128:(qc + 1) * 128, :], in_=ot[:])


def build_program(has_bias: bool, unroll: int = 1):
    key = (has_bias, unroll)
    if key in _PROGRAM_CACHE:
        return _PROGRAM_CACHE[key]
    nc = bass.Bass()
    hsT_d = nc.declare_dram_parameter("hsT", [HID, L], F32, isOutput=False)
    wT_d = nc.declare_dram_parameter("wT", [HID, 3 * FEAT], F32, isOutput=False)
    biasT_d = nc.declare_dram_parameter("biasT", [HPG, L, L], F32, isOutput=False)
    bvec_d = (nc.declare_dram_parameter("bvec", [1, 3 * FEAT], F32, isOutput=False)
              if has_bias else None)
    out_d = nc.declare_dram_parameter("out", [L, FEAT], F32, isOutput=True)
    with tile.TileContext(nc) as tc:
        for u in range(unroll):
            with ExitStack() as ctx:
                _emit_body(ctx, nc, tc, hsT_d, wT_d, biasT_d, out_d, bvec_d, u)
    _split_multiwaits(nc)
    _PROGRAM_CACHE[key] = nc
    return nc


def make_in_maps(hidden_states, Wqkv_w, Wqkv_b, bias, cu_seqlens, has_bias):
    """Host-side sharding/layout prep. Returns per-core input dicts."""
    scale = 1.0 / np.sqrt(D)
    in_maps = []
    for c in range(N_CORES):
        b, g = c // G, c % G
        lo, hi = int(cu_seqlens[b]), int(cu_seqlens[b + 1])
        hsT = np.ascontiguousarray(hidden_states[lo:hi].T)              # (768, 512)
        wq = Wqkv_w[g * FEAT:(g + 1) * FEAT] * scale                    # (384, 768)
        wk = Wqkv_w[DIM + g * FEAT:DIM + (g + 1) * FEAT]
        wv = Wqkv_w[2 * DIM + g * FEAT:2 * DIM + (g + 1) * FEAT]
        wT = np.ascontiguousarray(np.concatenate([wq, wk, wv], axis=0).T)  # (768, 1152)
        biasT = np.ascontiguousarray(
            bias[b, g * HPG:(g + 1) * HPG, :L, :L].transpose(0, 2, 1))  # (6, 512, 512)
        m = {"hsT": hsT, "wT": wT, "biasT": biasT}
        if has_bias:
            bq = Wqkv_b[g * FEAT:(g + 1) * FEAT] * scale
            bk = Wqkv_b[DIM + g * FEAT:DIM + (g + 1) * FEAT]
            bv = Wqkv_b[2 * DIM + g * FEAT:2 * DIM + (g + 1) * FEAT]
            m["bvec"] = np.concatenate([bq, bk, bv])[None, :].astype(np.float32)
        in_maps.append(m)
    return in_maps


def _structure_ok(cu_seqlens, indices, attn_mask, max_seqlen):
    try:
        if int(max_seqlen) != S:
            return False
        if cu_seqlens.shape != (B + 1,) or not np.array_equal(
                cu_seqlens, np.arange(B + 1) * L):
            return False
        exp_idx = (np.arange(B)[:, None] * S + np.arange(L)[None, :]).reshape(-1)
        if indices.shape != (B * L,) or not np.array_equal(indices, exp_idx):
            return False
        exp_mask = (np.arange(S)[None, :] < L).astype(attn_mask.dtype) * np.ones(
            (B, 1), attn_mask.dtype)
        if attn_mask.shape != (B, S) or not np.array_equal(attn_mask, exp_mask):
            return False
        return True
    except Exception:
        return False


def _numpy_fallback(hidden_states, Wqkv_w, Wqkv_b, bias, cu_seqlens,
                    max_seqlen_in_batch, indices, attn_mask):
    b = cu_seqlens.shape[0] - 1
    s = int(max_seqlen_in_batch)
    qkv = hidden_states @ Wqkv_w.T + Wqkv_b
    padded = np.zeros((b * s, 3 * DIM), dtype=qkv.dtype)
    padded[indices] = qkv
    qkv = padded.reshape(b, s, 3, H, D)
    q, k, v = qkv[:, :, 0], qkv[:, :, 1], qkv[:, :, 2]
    scores = np.einsum("bqhd,bkhd->bhqk", q, k) / np.sqrt(D) + bias
    scores = scores - scores.max(axis=-1, keepdims=True)
    e = np.exp(scores)
    p = e / e.sum(axis=-1, keepdims=True)
    attn = np.einsum("bhqk,bkhd->bqhd", p, v)
    return attn.reshape(b * s, H * D)[indices]


def kernel(hidden_states, Wqkv_w, Wqkv_b, bias, cu_seqlens,
           max_seqlen_in_batch, indices, attn_mask, _unroll=1):
    hidden_states = np.asarray(hidden_states, dtype=np.float32)
    Wqkv_w = np.asarray(Wqkv_w, dtype=np.float32)
    Wqkv_b = np.asarray(Wqkv_b, dtype=np.float32)
    bias = np.asarray(bias, dtype=np.float32)
    cu_seqlens = np.asarray(cu_seqlens)
    indices = np.asarray(indices)
    attn_mask = np.asarray(attn_mask)

    if (hidden_states.shape != (B * L, DIM) or Wqkv_w.shape != (3 * DIM, DIM)
            or bias.shape != (B, H, S, S)
            or not _structure_ok(cu_seqlens, indices, attn_mask,
                                 max_seqlen_in_batch)):
        return _numpy_fallback(hidden_states, Wqkv_w, Wqkv_b, bias, cu_seqlens,
                               max_seqlen_in_batch, indices, attn_mask)

    has_bias = bool(np.any(Wqkv_b != 0.0))
    nc = build_program(has_bias, unroll=_unroll)
    in_maps = make_in_maps(hidden_states, Wqkv_w, Wqkv_b, bias, cu_seqlens,
                           has_bias)
    res = run_bass_kernel_spmd(nc, in_maps, list(range(N_CORES)))
    out = np.empty((B * L, DIM), dtype=np.float32)
    for c in range(N_CORES):
        b, g = c // G, c % G
        out[b * L:(b + 1) * L, g * FEAT:(g + 1) * FEAT] = res.results[c]["out"]
    return out


# revision 5
# speedup vs baseline: 1.6048x; 1.6048x over previous
"""Bass/Trainium2 SPMD kernel for BertUnpadSelfAttentionWithExtras.

Problem shape (hardcoded, matches the grading reference):
  B=4 batches, S=1024 max seqlen, H=12 heads, D=64 head dim, DIM=768,
  L=512 real tokens per sequence (NNZ=2048 total).

Sharding over 8 cores: core c handles batch b = c//2 and head group
g = c%2 (6 heads each). Fully data-parallel, no collectives.

Key insight: padded key positions (>=512 within each sequence) have
k = v = 0 (scatter leaves them zero) and bias ~= -10000, so
exp(score - anything) underflows to exactly 0.0 in fp32 -> they
contribute nothing to softmax numerator or denominator. We therefore
compute attention over only the first 512 keys and read only
bias[:, :, :512, :512].

Device layout (per core):
  hsT  [768, 512]  : hidden states of this batch, transposed (host prep)
  wT   [768, 1152] : W^T columns for this head group: [q(384)|k(384)|v(384)],
                     q columns pre-scaled by 1/sqrt(64) (host prep)
  bvec [1, 1152]   : qkv bias slice (q part pre-scaled), only if nonzero
  biasT[6, 512, 512]: additive attn bias, transposed to [h, k, q] (host prep)
  out  [512, 384]  : output rows (tokens) x (6 heads * 64)

  qT/kT computed as [feat, tok] tiles -> directly usable as matmul
  lhsT/rhs for scoresT[k, q] = k @ qT. exp(scoresT) tiles are directly
  the lhsT for attn = probsT.T @ v_aug, where v_aug has a ones column
  per head giving the softmax denominator in the same PSUM tile.
"""

import numpy as np
from contextlib import ExitStack

import concourse.bass as bass
import concourse.mybir as mybir
import concourse.tile as tile
from concourse.bass_utils import run_bass_kernel_spmd

N_CORES = 8
B, S, H, D = 4, 1024, 12, 64
DIM = H * D          # 768
L = 512              # real tokens per sequence
G = 2                # head groups per batch
HPG = H // G         # 6 heads per group
FEAT = HPG * D       # 384 features per group
HID = DIM            # 768 contraction dim
KC = HID // 128      # 6 hidden chunks
TC = L // 128        # 4 token chunks
E = D + 2            # per-head column stride in v_aug / attn psum (even for fp32r)
F32 = mybir.dt.float32
F32R = mybir.dt.float32r
BF16 = mybir.dt.bfloat16

# dtype config: (projection/scores operand dtype, probs/v dtype, bias dma dtype)
VARIANTS = {
    "f32": (F32, F32, F32),
    "f32r": (F32R, F32R, F32),
    "f32r_bf16attn": (F32R, BF16, F32),
    "f32r_bf16attn_bf16bias": (F32R, BF16, BF16),
    "f32r_bf16bias": (F32R, F32R, BF16),
}
VARIANT = "f32"

_PROGRAM_CACHE: dict = {}


def _split_multiwaits(nc):
    """This walrus build rejects >1 sync wait per instruction; hoist all
    but the last wait onto single-wait NoOps preceding the instruction."""
    for f in nc.m.functions:
        for bb in f.blocks:
            insts = bb.instructions
            new = []
            changed = False
            for inst in insts:
                si = inst.sync_info
                waits = list(si.on_wait) if si and si.on_wait else []
                if len(waits) > 1:
                    changed = True
                    for j, w in enumerate(waits[:-1]):
                        new.append(mybir.InstNoOp(
                            name=f"{inst.name}-waitsplit-{j}",
                            engine=inst.engine,
                            sync_info=mybir.SyncInfo(on_wait=[w], on_update=[]),
                        ))
                    si.on_wait = [waits[-1]]
                new.append(inst)
            if changed:
                try:
                    bb.instructions = new
                except Exception:
                    insts.clear()
                    insts.extend(new)


def _emit_body(ctx, nc, tc, hsT_d, wT_d, biasT_d, out_d, bvec_d, uid, variant):
    Exp = mybir.ActivationFunctionType.Exp
    has_bias = bvec_d is not None
    MMDT, PDT, BDT = VARIANTS[variant]

    def mm(out, lhsT, rhs, start, stop):
        nc.tensor.matmul(out, lhsT=lhsT, rhs=rhs, start=start, stop=stop)

    def ms(ap, val):
        nc.vector.memset(ap.bitcast(F32) if ap.dtype == F32R else ap, val)

    pool = ctx.enter_context(tc.tile_pool(name=f"sb{uid}", bufs=1))
    bias_pool = ctx.enter_context(tc.tile_pool(name=f"bias{uid}", bufs=6))
    out_pool = ctx.enter_context(tc.tile_pool(name=f"out{uid}", bufs=3))
    psum_qkv = ctx.enter_context(tc.tile_pool(name=f"pq{uid}", bufs=3, space="PSUM"))
    psum_sc = ctx.enter_context(tc.tile_pool(name=f"ps{uid}", bufs=3, space="PSUM"))
    psum_at = ctx.enter_context(tc.tile_pool(name=f"pa{uid}", bufs=2, space="PSUM"))

    # --- load inputs ---
    wt = [pool.tile([128, 3 * FEAT], MMDT, tag=f"w{k}", name=f"w{k}") for k in range(KC)]
    for k in range(KC):
        nc.sync.dma_start(out=wt[k][:], in_=wT_d[k * 128:(k + 1) * 128, :])
    hst = [pool.tile([128, L], MMDT, tag=f"h{k}", name=f"h{k}") for k in range(KC)]
    for k in range(KC):
        nc.sync.dma_start(out=hst[k][:], in_=hsT_d[k * 128:(k + 1) * 128, :])
    if has_bias:
        bvec = pool.tile([1, 3 * FEAT], MMDT, tag="bvec", name="bvec")
        nc.sync.dma_start(out=bvec[:], in_=bvec_d[:])
        ones = pool.tile([1, L], MMDT, tag="ones", name="ones")
        ms(ones[:], 1.0)

    # --- QKV projection ---
    # qT/kT: [feat, tok] tiles (3 each of [128, 512]; 2 heads per tile)
    qkt = []  # [q0,q1,q2,k0,k1,k2]
    for which in range(2):  # 0=q, 1=k
        for m in range(FEAT // 128):
            ps = psum_qkv.tile([128, L], F32, tag="pqkv", name="pqkv")
            col0 = which * FEAT + m * 128
            for k in range(KC):
                mm(ps[:], wt[k][:, col0:col0 + 128], hst[k][:],
                   start=(k == 0), stop=(k == KC - 1 and not has_bias))
            if has_bias:
                mm(ps[:], bvec[0:1, col0:col0 + 128], ones[0:1, :],
                   start=False, stop=True)
            sb = pool.tile([128, L], MMDT, tag=f"qk{which}{m}", name=f"qk{which}{m}")
            nc.scalar.copy(sb[:], ps[:])
            qkt.append(sb)
    qt, kt = qkt[:3], qkt[3:]

    # v in natural [tok, feat] layout, interleaved with a ones column per
    # head: v_aug[t] is [128, 6*E], cols h*E..h*E+63 = v_h, col h*E+64 = 1,
    # col h*E+65 = 0 (pad so fp32r matmul dst offsets/sizes stay even)
    v_aug = []
    for t in range(TC):
        ps = psum_qkv.tile([128, FEAT], F32, tag="pqkv", name="pqkv_v")
        for k in range(KC):
            mm(ps[:], hst[k][:, t * 128:(t + 1) * 128],
               wt[k][:, 2 * FEAT:3 * FEAT],
               start=(k == 0), stop=(k == KC - 1 and not has_bias))
        if has_bias:
            mm(ps[:], ones[0:1, :128], bvec[0:1, 2 * FEAT:3 * FEAT],
               start=False, stop=True)
        va = pool.tile([128, HPG * E], PDT, tag=f"va{t}", name=f"va{t}")
        va3 = va[:].rearrange("p (h e) -> p h e", h=HPG)
        nc.vector.tensor_copy(
            va3[:, :, 0:D], ps[:].rearrange("p (h e) -> p h e", h=HPG))
        ms(va3[:, :, D:D + 2], 0.0)
        ms(va3[:, :, D:D + 1], 1.0)
        v_aug.append(va)

    # --- scoresT + softmax numerators ---
    # probs[h][kc]: [128(k), 512(q)] = exp(kT_chunk @ qT + biasT)
    probs = [[None] * TC for _ in range(HPG)]
    for h in range(HPG):
        ktile, part0 = kt[h // 2], (h % 2) * D
        qtile = qt[h // 2]
        for kc in range(TC):
            sc = psum_sc.tile([128, L], F32, tag="sc", name="sc")
            mm(sc[:],
               ktile[part0:part0 + D, kc * 128:(kc + 1) * 128],
               qtile[part0:part0 + D, :],
               start=True, stop=True)
            bt = bias_pool.tile([128, L], BDT, tag="bt", name="bt")
            nc.sync.dma_start(out=bt[:], in_=biasT_d[h, kc * 128:(kc + 1) * 128, :])
            nc.vector.tensor_add(sc[:], sc[:], bt[:])
            pr = pool.tile([128, L], PDT, tag=f"pr{h}_{kc}", name=f"pr{h}_{kc}")
            nc.scalar.activation(pr[:], sc[:], Exp)
            probs[h][kc] = pr

    # --- attention: out[q, h*64+d] = (probsT.T @ v_aug) / denom ---
    for qc in range(TC):
        at = psum_at.tile([128, HPG * E], F32, tag="at", name="at")
        for h in range(HPG):
            c0 = h * E
            for kc in range(TC):
                mm(at[:, c0:c0 + E],
                   probs[h][kc][:, qc * 128:(qc + 1) * 128],
                   v_aug[kc][:, c0:c0 + E],
                   start=(kc == 0), stop=(kc == TC - 1))
        rc = out_pool.tile([128, HPG], F32, tag="rc", name="rc")
        for h in range(HPG):
            nc.vector.reciprocal(rc[:, h:h + 1], at[:, h * E + D:h * E + D + 1])
        ot = out_pool.tile([128, FEAT], F32, tag="ot", name="ot")
        for h in range(HPG):
            nc.vector.tensor_scalar_mul(
                ot[:, h * D:(h + 1) * D], at[:, h * E:h * E + D],
                rc[:, h:h + 1])
        nc.sync.dma_start(out=out_d[qc * 128:(qc + 1) * 128, :], in_=ot[:])


def build_program(has_bias: bool, unroll: int = 1, variant: str | None = None):
    variant = variant or VARIANT
    key = (has_bias, unroll, variant)
    if key in _PROGRAM_CACHE:
        return _PROGRAM_CACHE[key]
    MMDT, PDT, BDT = VARIANTS[variant]
    nc = bass.Bass()
    hsT_d = nc.declare_dram_parameter("hsT", [HID, L], MMDT, isOutput=False)
    wT_d = nc.declare_dram_parameter("wT", [HID, 3 * FEAT], MMDT, isOutput=False)
    biasT_d = nc.declare_dram_parameter("biasT", [HPG, L, L], BDT, isOutput=False)
    bvec_d = (nc.declare_dram_parameter("bvec", [1, 3 * FEAT], MMDT, isOutput=False)
              if has_bias else None)
    out_d = nc.declare_dram_parameter("out", [L, FEAT], F32, isOutput=True)
    with tile.TileContext(nc) as tc:
        for u in range(unroll):
            with ExitStack() as ctx:
                _emit_body(ctx, nc, tc, hsT_d, wT_d, biasT_d, out_d, bvec_d, u,
                           variant)
    _split_multiwaits(nc)
    _PROGRAM_CACHE[key] = nc
    return nc


def make_in_maps(hidden_states, Wqkv_w, Wqkv_b, bias, cu_seqlens, has_bias,
                 variant=None):
    """Host-side sharding/layout prep. Returns per-core input dicts."""
    import ml_dtypes
    variant = variant or VARIANT
    bias_dt = ml_dtypes.bfloat16 if VARIANTS[variant][2] is BF16 else None
    scale = 1.0 / np.sqrt(D)
    in_maps = []
    for c in range(N_CORES):
        b, g = c // G, c % G
        lo, hi = int(cu_seqlens[b]), int(cu_seqlens[b + 1])
        hsT = np.ascontiguousarray(hidden_states[lo:hi].T)              # (768, 512)
        wq = Wqkv_w[g * FEAT:(g + 1) * FEAT] * scale                    # (384, 768)
        wk = Wqkv_w[DIM + g * FEAT:DIM + (g + 1) * FEAT]
        wv = Wqkv_w[2 * DIM + g * FEAT:2 * DIM + (g + 1) * FEAT]
        wT = np.ascontiguousarray(np.concatenate([wq, wk, wv], axis=0).T)  # (768, 1152)
        biasT = np.ascontiguousarray(
            bias[b, g * HPG:(g + 1) * HPG, :L, :L].transpose(0, 2, 1))  # (6, 512, 512)
        if bias_dt is not None:
            biasT = biasT.astype(bias_dt)
        m = {"hsT": hsT, "wT": wT, "biasT": biasT}
        if has_bias:
            bq = Wqkv_b[g * FEAT:(g + 1) * FEAT] * scale
            bk = Wqkv_b[DIM + g * FEAT:DIM + (g + 1) * FEAT]
            bv = Wqkv_b[2 * DIM + g * FEAT:2 * DIM + (g + 1) * FEAT]
            m["bvec"] = np.concatenate([bq, bk, bv])[None, :].astype(np.float32)
        in_maps.append(m)
    return in_maps


def _structure_ok(cu_seqlens, indices, attn_mask, max_seqlen):
    try:
        if int(max_seqlen) != S:
            return False
        if cu_seqlens.shape != (B + 1,) or not np.array_equal(
                cu_seqlens, np.arange(B + 1) * L):
            return False
        exp_idx = (np.arange(B)[:, None] * S + np.arange(L)[None, :]).reshape(-1)
        if indices.shape != (B * L,) or not np.array_equal(indices, exp_idx):
            return False
        exp_mask = (np.arange(S)[None, :] < L).astype(attn_mask.dtype) * np.ones(
            (B, 1), attn_mask.dtype)
        if attn_mask.shape != (B, S) or not np.array_equal(attn_mask, exp_mask):
            return False
        return True
    except Exception:
        return False


def _numpy_fallback(hidden_states, Wqkv_w, Wqkv_b, bias, cu_seqlens,
                    max_seqlen_in_batch, indices, attn_mask):
    b = cu_seqlens.shape[0] - 1
    s = int(max_seqlen_in_batch)
    qkv = hidden_states @ Wqkv_w.T + Wqkv_b
    padded = np.zeros((b * s, 3 * DIM), dtype=qkv.dtype)
    padded[indices] = qkv
    qkv = padded.reshape(b, s, 3, H, D)
    q, k, v = qkv[:, :, 0], qkv[:, :, 1], qkv[:, :, 2]
    scores = np.einsum("bqhd,bkhd->bhqk", q, k) / np.sqrt(D) + bias
    scores = scores - scores.max(axis=-1, keepdims=True)
    e = np.exp(scores)
    p = e / e.sum(axis=-1, keepdims=True)
    attn = np.einsum("bhqk,bkhd->bqhd", p, v)
    return attn.reshape(b * s, H * D)[indices]


def kernel(hidden_states, Wqkv_w, Wqkv_b, bias, cu_seqlens,
           max_seqlen_in_batch, indices, attn_mask, _unroll=1, _variant=None):
    hidden_states = np.asarray(hidden_states, dtype=np.float32)
    Wqkv_w = np.asarray(Wqkv_w, dtype=np.float32)
    Wqkv_b = np.asarray(Wqkv_b, dtype=np.float32)
    bias = np.asarray(bias, dtype=np.float32)
    cu_seqlens = np.asarray(cu_seqlens)
    indices = np.asarray(indices)
    attn_mask = np.asarray(attn_mask)

    if (hidden_states.shape != (B * L, DIM) or Wqkv_w.shape != (3 * DIM, DIM)
            or bias.shape != (B, H, S, S)
            or not _structure_ok(cu_seqlens, indices, attn_mask,
                                 max_seqlen_in_batch)):
        return _numpy_fallback(hidden_states, Wqkv_w, Wqkv_b, bias, cu_seqlens,
                               max_seqlen_in_batch, indices, attn_mask)

    has_bias = bool(np.any(Wqkv_b != 0.0))
    nc = build_program(has_bias, unroll=_unroll, variant=_variant)
    in_maps = make_in_maps(hidden_states, Wqkv_w, Wqkv_b, bias, cu_seqlens,
                           has_bias, variant=_variant)
    res = run_bass_kernel_spmd(nc, in_maps, list(range(N_CORES)))
    out = np.empty((B * L, DIM), dtype=np.float32)
    for c in range(N_CORES):
        b, g = c // G, c % G
        out[b * L:(b + 1) * L, g * FEAT:(g + 1) * FEAT] = res.results[c]["out"]
    return out


# revision 6
# speedup vs baseline: 2.7096x; 1.6885x over previous
"""Bass/Trainium2 SPMD kernel for BertUnpadSelfAttentionWithExtras.

Problem shape (hardcoded, matches the grading reference):
  B=4 batches, S=1024 max seqlen, H=12 heads, D=64 head dim, DIM=768,
  L=512 real tokens per sequence (NNZ=2048 total).

Sharding over 8 cores: core c handles batch b = c//2 and head group
g = c%2 (6 heads each). Fully data-parallel, no collectives.

Key insight: padded key positions (>=512 within each sequence) have
k = v = 0 (scatter leaves them zero) and bias ~= -10000, so
exp(score - anything) underflows to exactly 0.0 in fp32 -> they
contribute nothing to softmax numerator or denominator. We therefore
compute attention over only the first 512 keys and read only
bias[:, :, :512, :512].

Device layout (per core):
  hsT  [768, 512]  : hidden states of this batch, transposed (host prep)
  wT   [768, 1152] : W^T columns for this head group: [q(384)|k(384)|v(384)],
                     q columns pre-scaled by 1/sqrt(64) (host prep)
  bvec [1, 1152]   : qkv bias slice (q part pre-scaled), only if nonzero
  biasT[6, 512, 512]: additive attn bias, transposed to [h, k, q] (host prep)
  out  [512, 384]  : output rows (tokens) x (6 heads * 64)

  qT/kT computed as [feat, tok] tiles -> directly usable as matmul
  lhsT/rhs for scoresT[k, q] = k @ qT. exp(scoresT) tiles are directly
  the lhsT for attn = probsT.T @ v_aug, where v_aug has a ones column
  per head giving the softmax denominator in the same PSUM tile.
"""

import numpy as np
from contextlib import ExitStack

import concourse.bass as bass
import concourse.mybir as mybir
import concourse.tile as tile
from concourse.bass_utils import run_bass_kernel_spmd

N_CORES = 8
B, S, H, D = 4, 1024, 12, 64
DIM = H * D          # 768
L = 512              # real tokens per sequence
G = 2                # head groups per batch
HPG = H // G         # 6 heads per group
FEAT = HPG * D       # 384 features per group
HID = DIM            # 768 contraction dim
KC = HID // 128      # 6 hidden chunks
TC = L // 128        # 4 token chunks
E = D + 2            # per-head column stride in v_aug / attn psum (even for fp32r)
F32 = mybir.dt.float32
F32R = mybir.dt.float32r
BF16 = mybir.dt.bfloat16

# dtype config: (projection/scores operand dtype, probs/v dtype, bias dma dtype)
VARIANTS = {
    "f32": (F32, F32, F32),
    "f32r": (F32R, F32R, F32),
    "f32r_bf16attn": (F32R, BF16, F32),
    "f32r_bf16attn_bf16bias": (F32R, BF16, BF16),
    "f32r_bf16bias": (F32R, F32R, BF16),
}
VARIANT = "f32"

_PROGRAM_CACHE: dict = {}


def _split_multiwaits(nc):
    """This walrus build rejects >1 sync wait per instruction; hoist all
    but the last wait onto single-wait NoOps preceding the instruction."""
    for f in nc.m.functions:
        for bb in f.blocks:
            insts = bb.instructions
            new = []
            changed = False
            for inst in insts:
                si = inst.sync_info
                waits = list(si.on_wait) if si and si.on_wait else []
                if len(waits) > 1:
                    changed = True
                    for j, w in enumerate(waits[:-1]):
                        new.append(mybir.InstNoOp(
                            name=f"{inst.name}-waitsplit-{j}",
                            engine=inst.engine,
                            sync_info=mybir.SyncInfo(on_wait=[w], on_update=[]),
                        ))
                    si.on_wait = [waits[-1]]
                new.append(inst)
            if changed:
                try:
                    bb.instructions = new
                except Exception:
                    insts.clear()
                    insts.extend(new)


def _emit_body(ctx, nc, tc, hsT_d, wT_d, biasT_d, out_d, bvec_d, uid, variant):
    Exp = mybir.ActivationFunctionType.Exp
    has_bias = bvec_d is not None
    MMDT, PDT, BDT = VARIANTS[variant]

    def mm(out, lhsT, rhs, start, stop):
        nc.tensor.matmul(out, lhsT=lhsT, rhs=rhs, start=start, stop=stop)

    def ms(ap, val):
        nc.vector.memset(ap.bitcast(F32) if ap.dtype == F32R else ap, val)

    pool = ctx.enter_context(tc.tile_pool(name=f"sb{uid}", bufs=1))
    bias_pool = ctx.enter_context(tc.tile_pool(name=f"bias{uid}", bufs=14))
    out_pool = ctx.enter_context(tc.tile_pool(name=f"out{uid}", bufs=3))
    psum_qkv = ctx.enter_context(tc.tile_pool(name=f"pq{uid}", bufs=3, space="PSUM"))
    psum_sc = ctx.enter_context(tc.tile_pool(name=f"ps{uid}", bufs=3, space="PSUM"))
    psum_at = ctx.enter_context(tc.tile_pool(name=f"pa{uid}", bufs=2, space="PSUM"))

    # --- load inputs ---
    wt = [pool.tile([128, 3 * FEAT], MMDT, tag=f"w{k}", name=f"w{k}") for k in range(KC)]
    for k in range(KC):
        nc.sync.dma_start(out=wt[k][:], in_=wT_d[k * 128:(k + 1) * 128, :])
    hst = [pool.tile([128, L], MMDT, tag=f"h{k}", name=f"h{k}") for k in range(KC)]
    for k in range(KC):
        nc.sync.dma_start(out=hst[k][:], in_=hsT_d[k * 128:(k + 1) * 128, :])
    if has_bias:
        bvec = pool.tile([1, 3 * FEAT], MMDT, tag="bvec", name="bvec")
        nc.sync.dma_start(out=bvec[:], in_=bvec_d[:])
        ones = pool.tile([1, L], MMDT, tag="ones", name="ones")
        ms(ones[:], 1.0)

    # --- QKV projection ---
    # qT/kT: [feat, tok] tiles (3 each of [128, 512]; 2 heads per tile)
    qkt = []  # [q0,q1,q2,k0,k1,k2]
    for which in range(2):  # 0=q, 1=k
        for m in range(FEAT // 128):
            ps = psum_qkv.tile([128, L], F32, tag="pqkv", name="pqkv")
            col0 = which * FEAT + m * 128
            for k in range(KC):
                mm(ps[:], wt[k][:, col0:col0 + 128], hst[k][:],
                   start=(k == 0), stop=(k == KC - 1 and not has_bias))
            if has_bias:
                mm(ps[:], bvec[0:1, col0:col0 + 128], ones[0:1, :],
                   start=False, stop=True)
            sb = pool.tile([128, L], MMDT, tag=f"qk{which}{m}", name=f"qk{which}{m}")
            nc.scalar.copy(sb[:], ps[:])
            qkt.append(sb)
    qt, kt = qkt[:3], qkt[3:]

    # v in natural [tok, feat] layout, interleaved with a ones column per
    # head: v_aug[t] is [128, 6*E], cols h*E..h*E+63 = v_h, col h*E+64 = 1,
    # col h*E+65 = 0 (pad so fp32r matmul dst offsets/sizes stay even)
    v_aug = []
    for t in range(TC):
        ps = psum_qkv.tile([128, FEAT], F32, tag="pqkv", name="pqkv_v")
        for k in range(KC):
            mm(ps[:], hst[k][:, t * 128:(t + 1) * 128],
               wt[k][:, 2 * FEAT:3 * FEAT],
               start=(k == 0), stop=(k == KC - 1 and not has_bias))
        if has_bias:
            mm(ps[:], ones[0:1, :128], bvec[0:1, 2 * FEAT:3 * FEAT],
               start=False, stop=True)
        va = pool.tile([128, HPG * E], PDT, tag=f"va{t}", name=f"va{t}")
        va3 = va[:].rearrange("p (h e) -> p h e", h=HPG)
        nc.vector.tensor_copy(
            va3[:, :, 0:D], ps[:].rearrange("p (h e) -> p h e", h=HPG))
        ms(va3[:, :, D:D + 2], 0.0)
        ms(va3[:, :, D:D + 1], 1.0)
        v_aug.append(va)

    # --- scoresT + softmax numerators ---
    # probs[h][kc]: [128(k), 512(q)] = exp(kT_chunk @ qT + biasT)
    probs = [[None] * TC for _ in range(HPG)]
    for h in range(HPG):
        ktile, part0 = kt[h // 2], (h % 2) * D
        qtile = qt[h // 2]
        for kc in range(TC):
            sc = psum_sc.tile([128, L], F32, tag="sc", name="sc")
            mm(sc[:],
               ktile[part0:part0 + D, kc * 128:(kc + 1) * 128],
               qtile[part0:part0 + D, :],
               start=True, stop=True)
            bt = bias_pool.tile([128, L], BDT, tag="bt", name="bt")
            nc.sync.dma_start(out=bt[:], in_=biasT_d[h, kc * 128:(kc + 1) * 128, :])
            nc.vector.tensor_add(sc[:], sc[:], bt[:])
            pr = pool.tile([128, L], PDT, tag=f"pr{h}_{kc}", name=f"pr{h}_{kc}")
            nc.scalar.activation(pr[:], sc[:], Exp)
            probs[h][kc] = pr

    # --- attention: out[q, h*64+d] = (probsT.T @ v_aug) / denom ---
    for qc in range(TC):
        at = psum_at.tile([128, HPG * E], F32, tag="at", name="at")
        for h in range(HPG):
            c0 = h * E
            for kc in range(TC):
                mm(at[:, c0:c0 + E],
                   probs[h][kc][:, qc * 128:(qc + 1) * 128],
                   v_aug[kc][:, c0:c0 + E],
                   start=(kc == 0), stop=(kc == TC - 1))
        rc = out_pool.tile([128, HPG], F32, tag="rc", name="rc")
        for h in range(HPG):
            nc.vector.reciprocal(rc[:, h:h + 1], at[:, h * E + D:h * E + D + 1])
        ot = out_pool.tile([128, FEAT], F32, tag="ot", name="ot")
        for h in range(HPG):
            nc.vector.tensor_scalar_mul(
                ot[:, h * D:(h + 1) * D], at[:, h * E:h * E + D],
                rc[:, h:h + 1])
        nc.sync.dma_start(out=out_d[qc * 128:(qc + 1) * 128, :], in_=ot[:])


def build_program(has_bias: bool, unroll: int = 1, variant: str | None = None):
    variant = variant or VARIANT
    key = (has_bias, unroll, variant)
    if key in _PROGRAM_CACHE:
        return _PROGRAM_CACHE[key]
    MMDT, PDT, BDT = VARIANTS[variant]
    nc = bass.Bass()
    hsT_d = nc.declare_dram_parameter("hsT", [HID, L], MMDT, isOutput=False)
    wT_d = nc.declare_dram_parameter("wT", [HID, 3 * FEAT], MMDT, isOutput=False)
    biasT_d = nc.declare_dram_parameter("biasT", [HPG, L, L], BDT, isOutput=False)
    bvec_d = (nc.declare_dram_parameter("bvec", [1, 3 * FEAT], MMDT, isOutput=False)
              if has_bias else None)
    out_d = nc.declare_dram_parameter("out", [L, FEAT], F32, isOutput=True)
    with tile.TileContext(nc) as tc:
        for u in range(unroll):
            with ExitStack() as ctx:
                _emit_body(ctx, nc, tc, hsT_d, wT_d, biasT_d, out_d, bvec_d, u,
                           variant)
    _split_multiwaits(nc)
    _PROGRAM_CACHE[key] = nc
    return nc


def make_in_maps(hidden_states, Wqkv_w, Wqkv_b, bias, cu_seqlens, has_bias,
                 variant=None):
    """Host-side sharding/layout prep. Returns per-core input dicts."""
    import ml_dtypes
    variant = variant or VARIANT
    bias_dt = ml_dtypes.bfloat16 if VARIANTS[variant][2] is BF16 else None
    scale = 1.0 / np.sqrt(D)
    in_maps = []
    for c in range(N_CORES):
        b, g = c // G, c % G
        lo, hi = int(cu_seqlens[b]), int(cu_seqlens[b + 1])
        hsT = np.ascontiguousarray(hidden_states[lo:hi].T)              # (768, 512)
        wq = Wqkv_w[g * FEAT:(g + 1) * FEAT] * scale                    # (384, 768)
        wk = Wqkv_w[DIM + g * FEAT:DIM + (g + 1) * FEAT]
        wv = Wqkv_w[2 * DIM + g * FEAT:2 * DIM + (g + 1) * FEAT]
        wT = np.ascontiguousarray(np.concatenate([wq, wk, wv], axis=0).T)  # (768, 1152)
        biasT = np.ascontiguousarray(
            bias[b, g * HPG:(g + 1) * HPG, :L, :L].transpose(0, 2, 1))  # (6, 512, 512)
        if bias_dt is not None:
            biasT = biasT.astype(bias_dt)
        m = {"hsT": hsT, "wT": wT, "biasT": biasT}
        if has_bias:
            bq = Wqkv_b[g * FEAT:(g + 1) * FEAT] * scale
            bk = Wqkv_b[DIM + g * FEAT:DIM + (g + 1) * FEAT]
            bv = Wqkv_b[2 * DIM + g * FEAT:2 * DIM + (g + 1) * FEAT]
            m["bvec"] = np.concatenate([bq, bk, bv])[None, :].astype(np.float32)
        in_maps.append(m)
    return in_maps


def _structure_ok(cu_seqlens, indices, attn_mask, max_seqlen):
    try:
        if int(max_seqlen) != S:
            return False
        if cu_seqlens.shape != (B + 1,) or not np.array_equal(
                cu_seqlens, np.arange(B + 1) * L):
            return False
        exp_idx = (np.arange(B)[:, None] * S + np.arange(L)[None, :]).reshape(-1)
        if indices.shape != (B * L,) or not np.array_equal(indices, exp_idx):
            return False
        exp_mask = (np.arange(S)[None, :] < L).astype(attn_mask.dtype) * np.ones(
            (B, 1), attn_mask.dtype)
        if attn_mask.shape != (B, S) or not np.array_equal(attn_mask, exp_mask):
            return False
        return True
    except Exception:
        return False


def _numpy_fallback(hidden_states, Wqkv_w, Wqkv_b, bias, cu_seqlens,
                    max_seqlen_in_batch, indices, attn_mask):
    b = cu_seqlens.shape[0] - 1
    s = int(max_seqlen_in_batch)
    qkv = hidden_states @ Wqkv_w.T + Wqkv_b
    padded = np.zeros((b * s, 3 * DIM), dtype=qkv.dtype)
    padded[indices] = qkv
    qkv = padded.reshape(b, s, 3, H, D)
    q, k, v = qkv[:, :, 0], qkv[:, :, 1], qkv[:, :, 2]
    scores = np.einsum("bqhd,bkhd->bhqk", q, k) / np.sqrt(D) + bias
    scores = scores - scores.max(axis=-1, keepdims=True)
    e = np.exp(scores)
    p = e / e.sum(axis=-1, keepdims=True)
    attn = np.einsum("bhqk,bkhd->bqhd", p, v)
    return attn.reshape(b * s, H * D)[indices]


def kernel(hidden_states, Wqkv_w, Wqkv_b, bias, cu_seqlens,
           max_seqlen_in_batch, indices, attn_mask, _unroll=1, _variant=None):
    hidden_states = np.asarray(hidden_states, dtype=np.float32)
    Wqkv_w = np.asarray(Wqkv_w, dtype=np.float32)
    Wqkv_b = np.asarray(Wqkv_b, dtype=np.float32)
    bias = np.asarray(bias, dtype=np.float32)
    cu_seqlens = np.asarray(cu_seqlens)
    indices = np.asarray(indices)
    attn_mask = np.asarray(attn_mask)

    if (hidden_states.shape != (B * L, DIM) or Wqkv_w.shape != (3 * DIM, DIM)
            or bias.shape != (B, H, S, S)
            or not _structure_ok(cu_seqlens, indices, attn_mask,
                                 max_seqlen_in_batch)):
        return _numpy_fallback(hidden_states, Wqkv_w, Wqkv_b, bias, cu_seqlens,
                               max_seqlen_in_batch, indices, attn_mask)

    has_bias = bool(np.any(Wqkv_b != 0.0))
    nc = build_program(has_bias, unroll=_unroll, variant=_variant)
    in_maps = make_in_maps(hidden_states, Wqkv_w, Wqkv_b, bias, cu_seqlens,
                           has_bias, variant=_variant)
    res = run_bass_kernel_spmd(nc, in_maps, list(range(N_CORES)))
    out = np.empty((B * L, DIM), dtype=np.float32)
    for c in range(N_CORES):
        b, g = c // G, c % G
        out[b * L:(b + 1) * L, g * FEAT:(g + 1) * FEAT] = res.results[c]["out"]
    return out
